# revision 3
# baseline (speedup 1.0000x reference)
"""MobiuAttention Trainium2 kernel (8 NeuronCores, SPMD).

Sharding: core i handles (batch b = i//2, head-group g = i%2) -> 8 local heads.
Per core: fp32r projections, complexity sensor, chunked linear-attention
recurrence (chunk C=128, log-space cumulative decay, head-PAIR packed on the
128 partitions, fp32 matmuls), o_proj partial with the local head-slice of
o_w. Host sums the two partial y's per batch.
"""
import sys
sys.path.insert(0, '/opt/trn_rl_repo')

import numpy as np
import bass_rust
import concourse.bass as bass
import concourse.mybir as mybir
import concourse.tile as tile
from concourse.bass_utils import run_bass_kernel_spmd
from concourse.masks import make_identity, make_upper_triangular

F32 = mybir.dt.float32
F32R = mybir.dt.float32r
U8 = mybir.dt.uint8
BF16 = mybir.dt.bfloat16
AL = mybir.AluOpType
AF = mybir.ActivationFunctionType

B, T, D, H, E = 4, 2048, 1024, 16, 64
DH = D // 4          # 256 sensor hidden
HL = 8               # heads per core
NP = HL // 2         # 4 head pairs
DL = HL * E          # 512 local head dim
SC = 8               # superchunks
TC = T // SC         # 256 tokens per superchunk
C = 128              # recurrence chunk
NT = TC // C         # 2 chunks per superchunk
NDT = D // 128       # 8 contraction tiles
LOGCLIP = float(np.log(0.9995))

SEQ_ENGINES = {mybir.EngineType.PE, mybir.EngineType.DVE, mybir.EngineType.Activation,
               mybir.EngineType.Pool, mybir.EngineType.SP}


def _split_multiwait(nc, max_waits=1):
    """Walrus here encodes at most one sync-wait per instruction; hoist extra
    waits onto single-wait NOPs just before, on the same in-order sequencer."""
    for f in nc.m.functions:
        for bb in f.blocks:
            changed = False
            newlist = []
            for inst in bb.instructions:
                si = inst.sync_info
                if (si is not None and len(si.on_wait) > max_waits
                        and inst.engine in SEQ_ENGINES):
                    waits = list(si.on_wait)
                    for w in waits[:-1]:
                        nop = mybir.InstNoOp(name=nc.get_next_instruction_name(),
                                             ins=[], outs=[])
                        nop.engine = inst.engine
                        nop.sync_info = bass_rust.SyncInfo(on_wait=[w], on_update=[])
                        newlist.append(nop)
                        nc.register_instruction(nop)
                    inst.sync_info = bass_rust.SyncInfo(
                        on_wait=[waits[-1]], on_update=list(si.on_update))
                    changed = True
                newlist.append(inst)
            if changed:
                bb.instructions = newlist


def _build():
    nc = bass.Bass(trn_type="TRN2", num_devices=8)
    xT_d = nc.dram_tensor("xT", [128, NDT * T], F32R, kind="ExternalInput")
    wq_d = nc.dram_tensor("wq", [128, NDT * DL], F32R, kind="ExternalInput")
    wk_d = nc.dram_tensor("wk", [128, NDT * DL], F32R, kind="ExternalInput")
    wv_d = nc.dram_tensor("wv", [128, NDT * DL], F32R, kind="ExternalInput")
    wo_d = nc.dram_tensor("wo", [128, 4 * D], F32R, kind="ExternalInput")
    cs1_d = nc.dram_tensor("cs1", [128, NDT * DH], F32R, kind="ExternalInput")
    cs2_d = nc.dram_tensor("cs2", [128, 2 * HL], F32R, kind="ExternalInput")
    b1_d = nc.dram_tensor("b1", [128, 2], F32, kind="ExternalInput")
    b2_d = nc.dram_tensor("b2", [128, HL], F32, kind="ExternalInput")
    lb_d = nc.dram_tensor("lb", [128, DL], F32, kind="ExternalInput")
    y_d = nc.dram_tensor("y", [T, D], F32, kind="ExternalOutput")

    with tile.TileContext(nc) as tc:
        with tc.tile_pool(name="wpool", bufs=1) as wpool, \
             tc.tile_pool(name="cpool", bufs=1) as cpool, \
             tc.tile_pool(name="state", bufs=1) as state, \
             tc.tile_pool(name="xpool", bufs=2) as xpool, \
             tc.tile_pool(name="qkv", bufs=2) as qkv, \
             tc.tile_pool(name="hpool", bufs=2) as hpool, \
             tc.tile_pool(name="upool", bufs=2) as upool, \
             tc.tile_pool(name="otpool", bufs=2) as otpool, \
             tc.tile_pool(name="ypool", bufs=2) as ypool, \
             tc.tile_pool(name="rec", bufs=3) as rec, \
             tc.tile_pool(name="small", bufs=4) as small, \
             tc.tile_pool(name="psA", bufs=1, space="PSUM") as psA, \
             tc.tile_pool(name="psB", bufs=2, space="PSUM") as psB:

            # ---- constants ----
            ident = cpool.tile([128, 128], F32)
            make_identity(nc, ident[:])
            tri = cpool.tile([128, 128], F32)
            make_upper_triangular(nc, tri[:], val=1.0, diag=True)
            tri_u8 = cpool.tile([128, 128], U8)
            nc.vector.tensor_copy(tri_u8[:], tri[:])
            z128 = cpool.tile([128, 128], F32)
            nc.vector.memset(z128[:], 0.0)
            z128b = cpool.tile([128, 128], BF16)
            nc.vector.memset(z128b[:], 0.0)

            # ---- weights ----
            wq = wpool.tile([128, NDT * DL], F32R)
            nc.sync.dma_start(wq[:], wq_d[:])
            wk = wpool.tile([128, NDT * DL], F32R)
            nc.sync.dma_start(wk[:], wk_d[:])
            wv = wpool.tile([128, NDT * DL], F32R)
            nc.sync.dma_start(wv[:], wv_d[:])
            wo = wpool.tile([128, 4 * D], F32R)
            nc.sync.dma_start(wo[:], wo_d[:])
            cs1 = wpool.tile([128, NDT * DH], F32R)
            nc.sync.dma_start(cs1[:], cs1_d[:])
            cs2 = wpool.tile([128, 2 * HL], F32R)
            nc.sync.dma_start(cs2[:], cs2_d[:])
            b1 = wpool.tile([128, 2], F32)
            nc.sync.dma_start(b1[:], b1_d[:])
            b2 = wpool.tile([128, HL], F32)
            nc.sync.dma_start(b2[:], b2_d[:])
            lb = wpool.tile([128, DL], F32)
            nc.sync.dma_start(lb[:], lb_d[:])

            # ---- per-pair recurrent state [ (h0 e | h1 e), f ] ----
            S = []
            for mo in range(NP):
                sh = state.tile([128, 64], F32, tag=f"S{mo}", name=f"S{mo}")
                nc.vector.memset(sh[:], 0.0)
                S.append(sh)

            for sc in range(SC):
                xt = xpool.tile([128, NDT * TC], F32R, tag="xt")
                for dt in range(NDT):
                    nc.sync.dma_start(
                        xt[:, dt * TC:(dt + 1) * TC],
                        xT_d[:, dt * T + sc * TC: dt * T + sc * TC + TC])

                # ---- Q,K -> per-pair [ (2x64 e), t(TC) ] ----
                q_et, k_et = [], []
                for name, w, dst in (("q", wq, q_et), ("k", wk, k_et)):
                    for mo in range(NP):
                        pp = psB.tile([128, TC], F32, tag="proj")
                        for dt in range(NDT):
                            nc.tensor.matmul(
                                pp[:],
                                w[:, dt * DL + mo * 128: dt * DL + (mo + 1) * 128],
                                xt[:, dt * TC:(dt + 1) * TC],
                                start=(dt == 0), stop=(dt == NDT - 1))
                        sb = qkv.tile([128, TC], F32, tag=f"{name}{mo}")
                        nc.vector.tensor_copy(sb[:], pp[:])
                        dst.append(sb)

                # ---- V -> [t(128 x NT), dout(DL)] ----
                v_te, v_bf = [], []
                for tt in range(NT):
                    pp = psB.tile([128, DL], F32, tag="proj")
                    for dt in range(NDT):
                        nc.tensor.matmul(
                            pp[:, 0:DL],
                            xt[:, dt * TC + tt * 128: dt * TC + (tt + 1) * 128],
                            wv[:, dt * DL:(dt + 1) * DL],
                            start=(dt == 0), stop=(dt == NDT - 1))
                    sb = qkv.tile([128, DL], F32, tag=f"v{tt}")
                    nc.vector.tensor_copy(sb[:], pp[:, 0:DL])
                    v_te.append(sb)
                    vb = qkv.tile([128, DL], BF16, tag=f"vb{tt}")
                    nc.vector.tensor_copy(vb[:], sb[:])
                    v_bf.append(vb)

                # ---- sensor ----
                hid = []
                for mo in range(2):
                    pp = psB.tile([128, TC], F32, tag="proj")
                    for dt in range(NDT):
                        nc.tensor.matmul(
                            pp[:],
                            cs1[:, dt * DH + mo * 128: dt * DH + (mo + 1) * 128],
                            xt[:, dt * TC:(dt + 1) * TC],
                            start=(dt == 0), stop=(dt == NDT - 1))
                    sb = hpool.tile([128, TC], F32R, tag=f"h{mo}")
                    nc.scalar.activation(sb[:], pp[:], AF.Tanh, bias=b1[:, mo:mo + 1])
                    hid.append(sb)

                u_tt = []
                for tt in range(NT):
                    pp = psA.tile([128, HL], F32, tag="lamT")
                    for k2 in range(2):
                        nc.tensor.matmul(
                            pp[:],
                            hid[k2][:, tt * 128:(tt + 1) * 128],
                            cs2[:, k2 * HL:(k2 + 1) * HL],
                            start=(k2 == 0), stop=(k2 == 1))
                    zb = small.tile([128, HL], F32, tag="zb")
                    nc.vector.tensor_add(zb[:], pp[:], b2[:])
                    lc = small.tile([128, HL], F32, tag="lc")
                    nc.scalar.activation(lc[:], zb[:], AF.Sigmoid)
                    uu = upool.tile([128, HL], F32, tag=f"u{tt}")
                    nc.scalar.activation(uu[:], lc[:], AF.Ln, bias=1.0, scale=0.2)
                    u_tt.append(uu)

                # ---- OT collector: [64 f, (h-local, t)] per pair ----
                OT = [otpool.tile([128, TC], F32R, tag=f"ot{mo}",
                                  name=f"OT{mo}_{sc}") for mo in range(NP)]

                # ---- recurrence: chunk x pair ----
                for tt in range(NT):
                    for mo in range(NP):
                        q_p = q_et[mo][:, tt * 128:(tt + 1) * 128]
                        k_p = k_et[mo][:, tt * 128:(tt + 1) * 128]
                        v_p = v_te[tt][:, mo * 128:(mo + 1) * 128]
                        vbf_p = v_bf[tt][:, mo * 128:(mo + 1) * 128]
                        # log-lambda [t, (2x64 e)] then transpose to pair-et
                        lam = rec.tile([128, 128], F32, tag="lam")
                        for j in range(2):
                            h = 2 * mo + j
                            nc.vector.tensor_scalar(
                                lam[:, j * 64:(j + 1) * 64],
                                lb[:, h * 64:(h + 1) * 64],
                                u_tt[tt][:, h:h + 1], LOGCLIP,
                                AL.add, AL.min)
                        lamT = psA.tile([128, 128], F32, tag="lamT")
                        nc.tensor.transpose(lamT[:], lam[:], ident[:])
                        L = rec.tile([128, 128], F32, tag="L")
                        nc.vector.tensor_tensor_scan(
                            L[:], lamT[:], z128[:], 0.0, AL.add, AL.add)

                        L127 = L[:, 127:128]
                        ccol = small.tile([128, 1], F32, tag="ccol")
                        nc.vector.tensor_scalar_mul(ccol[:], L127, 0.5)
                        cneg = small.tile([128, 1], F32, tag="cneg")
                        nc.vector.tensor_scalar_mul(cneg[:], L127, -0.5)
                        ec = small.tile([128, 1], F32, tag="ec")
                        nc.scalar.activation(ec[:], L127, AF.Exp, scale=0.5)
                        aend = small.tile([128, 1], F32, tag="aend")
                        nc.scalar.activation(aend[:], L127, AF.Exp)

                        eq = rec.tile([128, 128], F32, tag="eq")
                        nc.scalar.activation(eq[:], L[:], AF.Exp, bias=cneg[:])
                        ekc = rec.tile([128, 128], F32, tag="ekc")
                        nc.scalar.activation(ekc[:], L[:], AF.Exp, bias=ccol[:],
                                             scale=-1.0)
                        ek7 = rec.tile([128, 128], F32, tag="ek7")
                        nc.scalar.activation(ek7[:], L[:], AF.Exp, bias=L127,
                                             scale=-1.0)

                        qt = rec.tile([128, 128], BF16, tag="qt")
                        nc.vector.tensor_mul(qt[:], q_p, eq[:])
                        kt = rec.tile([128, 128], BF16, tag="kt")
                        nc.vector.tensor_mul(kt[:], k_p, ekc[:])
                        kh = rec.tile([128, 128], F32, tag="kh")
                        nc.vector.tensor_mul(kh[:], k_p, ek7[:])

                        # K-hat pair transpose -> [t, (2x64 e)]
                        khT = psA.tile([128, 128], F32, tag="khT")
                        nc.tensor.transpose(khT[:], kh[:], ident[:])
                        khTs = rec.tile([128, 128], F32, tag="khTs")
                        nc.vector.tensor_copy(khTs[:], khT[:])

                        # S_scaled (both heads)
                        ssc = rec.tile([128, 64], BF16, tag="ssc")
                        nc.vector.tensor_scalar_mul(ssc[:], S[mo][:], ec[:])

                        # state delta for the pair (block-diagonal valid)
                        sd = psA.tile([128, 128], F32, tag="sd")
                        nc.tensor.matmul(sd[:], khTs[:], v_p, start=True, stop=True)

                        op = psB.tile([128, 128], F32, tag="outT")
                        for j in range(2):
                            sl = slice(j * 64, (j + 1) * 64)
                            # intra-chunk attention for head h = 2*mo + j
                            at = psA.tile([128, 128], F32, tag="at")
                            nc.tensor.matmul(at[:], kt[sl, :], qt[sl, :],
                                             start=True, stop=True)
                            atm = rec.tile([128, 128], BF16, tag="atm")
                            nc.gpsimd.tensor_copy(atm[:], z128b[:])
                            nc.vector.copy_predicated(atm[:], tri_u8[:], at[:])

                            nc.tensor.matmul(op[sl, :],
                                             vbf_p[:, j * 64:(j + 1) * 64],
                                             atm[:], start=True, stop=False)
                            nc.tensor.matmul(op[sl, :], ssc[sl, :], qt[sl, :],
                                             start=False, stop=True)
                            # state update for head h
                            nc.vector.scalar_tensor_tensor(
                                S[mo][sl, :], S[mo][sl, :], aend[sl, :],
                                sd[sl, j * 64:(j + 1) * 64], AL.mult, AL.add)
                        nc.vector.tensor_copy(
                            OT[mo][:, tt * 128:(tt + 1) * 128], op[:])

                # ---- o_proj ----
                for tt in range(NT):
                    for no in range(2):
                        pp = psB.tile([128, 512], F32, tag="proj")
                        for mo in range(NP):
                            nc.tensor.matmul(
                                pp[:],
                                OT[mo][:, tt * 128:(tt + 1) * 128],
                                wo[:, mo * D + no * 512: mo * D + no * 512 + 512],
                                start=(mo == 0), stop=(mo == NP - 1))
                        ysb = ypool.tile([128, 512], F32, tag="y")
                        nc.vector.tensor_copy(ysb[:], pp[:])
                        nc.sync.dma_start(
                            y_d[sc * TC + tt * 128: sc * TC + (tt + 1) * 128,
                                no * 512:(no + 1) * 512],
                            ysb[:])
    _split_multiwait(nc)
    return nc


_NC = None
LAST = None  # last BassKernelResults (exec_time_ns, trace path) for test harness

def _get_nc():
    global _NC
    if _NC is None:
        _NC = _build()
    return _NC


def _sigmoid(x):
    return 1.0 / (1.0 + np.exp(-x))


def kernel(x, q_w, k_w, v_w, o_w, cs_w1, cs_b1, cs_w2, cs_b2, decay_params):
    x = np.asarray(x, np.float32)
    nc = _get_nc()

    def wlay(wT_cols):  # [1024, M] -> [128, 8*M] (dt-major along free)
        return np.ascontiguousarray(
            wT_cols.reshape(NDT, 128, wT_cols.shape[1]).transpose(1, 0, 2)
            .reshape(128, -1))

    qwT = np.asarray(q_w, np.float32).T
    kwT = np.asarray(k_w, np.float32).T
    vwT = np.asarray(v_w, np.float32).T
    owT = np.asarray(o_w, np.float32).T
    cs1T = np.asarray(cs_w1, np.float32).T      # [1024, 256]
    cs2T = np.asarray(cs_w2, np.float32).T      # [256, 16]
    lbase = np.log(_sigmoid(np.asarray(decay_params, np.float32)))  # [H, E]
    b1c = np.ascontiguousarray(np.asarray(cs_b1, np.float32).reshape(2, 128).T)

    in_maps = []
    for i in range(8):
        b, g = i // 2, i % 2
        hs = g * HL
        xT = x[b].T                                            # [1024, 2048]
        xTl = np.ascontiguousarray(
            xT.reshape(NDT, 128, T).transpose(1, 0, 2).reshape(128, NDT * T))
        wo_loc = owT[hs * E:(hs + HL) * E, :]                  # [512, 1024]
        wol = np.ascontiguousarray(                            # [128, 4*1024]
            wo_loc.reshape(4, 128, D).transpose(1, 0, 2).reshape(128, 4 * D))
        cs2l = np.ascontiguousarray(
            cs2T[:, hs:hs + HL].reshape(2, 128, HL).transpose(1, 0, 2)
            .reshape(128, 2 * HL))
        in_maps.append({
            "xT": xTl,
            "wq": wlay(qwT[:, hs * E:(hs + HL) * E]),
            "wk": wlay(kwT[:, hs * E:(hs + HL) * E]),
            "wv": wlay(vwT[:, hs * E:(hs + HL) * E]),
            "wo": wol,
            "cs1": wlay(cs1T),
            "cs2": cs2l,
            "b1": b1c,
            "b2": np.ascontiguousarray(
                np.broadcast_to(np.asarray(cs_b2, np.float32)[hs:hs + HL],
                                (128, HL))),
            "lb": np.ascontiguousarray(
                np.broadcast_to(lbase[hs:hs + HL].reshape(1, DL), (128, DL))),
        })

    res = run_bass_kernel_spmd(nc, in_maps, core_ids=list(range(8)))
    global LAST
    LAST = res
    y = np.empty((B, T, D), np.float32)
    for b in range(B):
        y[b] = res.results[2 * b]["y"] + res.results[2 * b + 1]["y"]
    return y



# revision 17
# speedup vs baseline: 1.2668x; 1.2668x over previous
"""MobiuAttention Trainium2 kernel (8 NeuronCores, SPMD).

Sharding: core i handles (batch b = i//2, head-group g = i%2) -> 8 local heads.
Per core: bf16 projections, one-time complexity sensor (activation functions
grouped to avoid ACT_TABLE_LOAD churn), chunked linear-attention recurrence
(chunk C=128, log-space cumulative decay, head-PAIR packed on 128 partitions,
bf16 matmuls), o_proj partial with the local head-slice of o_w. Host sums the
two partial y's per batch.

Emission is software-pipelined: recurrence elementwise for superchunk sc-1 is
interleaved with the projection matmul burst for sc, keeping the PE dense so
the HAM clock gate stays at full rate.
"""
import sys
sys.path.insert(0, '/opt/trn_rl_repo')

import numpy as np
import bass_rust
import concourse.bass as bass
import concourse.mybir as mybir
import concourse.tile as tile
from concourse.bass_utils import run_bass_kernel_spmd
from concourse.masks import make_identity, make_upper_triangular

F32 = mybir.dt.float32
F32R = mybir.dt.float32r
BF16 = mybir.dt.bfloat16
AL = mybir.AluOpType
AF = mybir.ActivationFunctionType

B, T, D, H, E = 4, 2048, 1024, 16, 64
DH = D // 4          # 256 sensor hidden
HL = 8               # heads per core
NP = HL // 2         # 4 head pairs
DL = HL * E          # 512 local head dim
SC = 8               # superchunks
TC = T // SC         # 256 tokens per superchunk
C = 128              # recurrence chunk
NT = TC // C         # 2 chunks per superchunk
NDT = D // 128       # 8 contraction tiles
LOGCLIP = float(np.log(0.9995))

SEQ_ENGINES = {mybir.EngineType.PE, mybir.EngineType.DVE, mybir.EngineType.Activation,
               mybir.EngineType.Pool, mybir.EngineType.SP}


def _split_multiwait(nc, max_waits=1):
    """Walrus here encodes at most one sync-wait per instruction; hoist extra
    waits onto single-wait NOPs just before, on the same in-order sequencer."""
    for f in nc.m.functions:
        for bb in f.blocks:
            changed = False
            newlist = []
            for inst in bb.instructions:
                si = inst.sync_info
                if (si is not None and len(si.on_wait) > max_waits
                        and inst.engine in SEQ_ENGINES):
                    waits = list(si.on_wait)
                    for w in waits[:-1]:
                        nop = mybir.InstNoOp(name=nc.get_next_instruction_name(),
                                             ins=[], outs=[])
                        nop.engine = inst.engine
                        nop.sync_info = bass_rust.SyncInfo(on_wait=[w], on_update=[])
                        newlist.append(nop)
                        nc.register_instruction(nop)
                    inst.sync_info = bass_rust.SyncInfo(
                        on_wait=[waits[-1]], on_update=list(si.on_update))
                    changed = True
                newlist.append(inst)
            if changed:
                bb.instructions = newlist


def _build():
    nc = bass.Bass(trn_type="TRN2", num_devices=8)
    xT_d = nc.dram_tensor("xT", [128, NDT * T], BF16, kind="ExternalInput")
    wq_d = nc.dram_tensor("wq", [128, NDT * DL], BF16, kind="ExternalInput")
    wk_d = nc.dram_tensor("wk", [128, NDT * DL], BF16, kind="ExternalInput")
    wv_d = nc.dram_tensor("wv", [128, NDT * DL], BF16, kind="ExternalInput")
    wo_d = nc.dram_tensor("wo", [128, 4 * D], BF16, kind="ExternalInput")
    cs1_d = nc.dram_tensor("cs1", [128, NDT * DH], BF16, kind="ExternalInput")
    cs2_d = nc.dram_tensor("cs2", [128, 2 * HL], BF16, kind="ExternalInput")
    b1_d = nc.dram_tensor("b1", [128, 2], F32, kind="ExternalInput")
    b2_d = nc.dram_tensor("b2", [128, HL], F32, kind="ExternalInput")
    lb_d = nc.dram_tensor("lb", [128, DL], F32, kind="ExternalInput")
    y_d = nc.dram_tensor("y", [T, D], F32, kind="ExternalOutput")

    with tile.TileContext(nc) as tc:
        with tc.tile_pool(name="wpool", bufs=1) as wpool, \
             tc.tile_pool(name="cpool", bufs=1) as cpool, \
             tc.tile_pool(name="state", bufs=1) as state, \
             tc.tile_pool(name="xpool", bufs=1) as xpool, \
             tc.tile_pool(name="hpool", bufs=1) as hpool, \
             tc.tile_pool(name="upool", bufs=1) as upool, \
             tc.tile_pool(name="qkv", bufs=2) as qkv, \
             tc.tile_pool(name="otpool", bufs=2) as otpool, \
             tc.tile_pool(name="ypool", bufs=2) as ypool, \
             tc.tile_pool(name="rec", bufs=3) as rec, \
             tc.tile_pool(name="mid", bufs=2) as midp, \
             tc.tile_pool(name="small", bufs=4) as small, \
             tc.tile_pool(name="psT", bufs=2, space="PSUM") as psT, \
             tc.tile_pool(name="psSD", bufs=1, space="PSUM") as psSD, \
             tc.tile_pool(name="psAT", bufs=2, space="PSUM") as psAT, \
             tc.tile_pool(name="psOP", bufs=1, space="PSUM") as psOP, \
             tc.tile_pool(name="psB", bufs=2, space="PSUM") as psB:

            # ---- constants ----
            identf = cpool.tile([128, 128], F32)
            make_identity(nc, identf[:])
            tri = cpool.tile([128, 128], F32)
            make_upper_triangular(nc, tri[:], val=1.0, diag=True)
            tri_u8 = cpool.tile([128, 128], mybir.dt.uint8)
            nc.vector.tensor_copy(tri_u8[:], tri[:])
            z128 = cpool.tile([128, 128], F32)
            nc.vector.memset(z128[:], 0.0)
            z128b = cpool.tile([128, 128], BF16)
            nc.vector.memset(z128b[:], 0.0)

            # ---- weights ----
            wq = wpool.tile([128, NDT * DL], BF16)
            nc.sync.dma_start(wq[:], wq_d[:])
            wk = wpool.tile([128, NDT * DL], BF16)
            nc.sync.dma_start(wk[:], wk_d[:])
            wv = wpool.tile([128, NDT * DL], BF16)
            nc.sync.dma_start(wv[:], wv_d[:])
            wo = wpool.tile([128, 4 * D], BF16)
            nc.sync.dma_start(wo[:], wo_d[:])
            cs1 = wpool.tile([128, NDT * DH], BF16)
            nc.sync.dma_start(cs1[:], cs1_d[:])
            cs2 = wpool.tile([128, 2 * HL], BF16)
            nc.sync.dma_start(cs2[:], cs2_d[:])
            b1 = wpool.tile([128, 2], F32)
            nc.sync.dma_start(b1[:], b1_d[:])
            b2 = wpool.tile([128, HL], F32)
            nc.sync.dma_start(b2[:], b2_d[:])
            lb = wpool.tile([128, DL], F32)
            nc.sync.dma_start(lb[:], lb_d[:])

            # ---- x for all T, upfront ----
            xt = xpool.tile([128, NDT * T], BF16, name="xt")
            for dt in range(NDT):
                nc.sync.dma_start(xt[:, dt * T:(dt + 1) * T],
                                  xT_d[:, dt * T:(dt + 1) * T])

            def xsl(sc, dt):
                return xt[:, dt * T + sc * TC: dt * T + sc * TC + TC]

            # ---- per-pair recurrent state [ (h0 e | h1 e), f ] ----
            S = []
            for mo in range(NP):
                sh = state.tile([128, 64], F32, tag=f"S{mo}", name=f"S{mo}")
                nc.vector.memset(sh[:], 0.0)
                S.append(sh)

            # =========== one-time sensor phase (grouped by Act function) ====
            # hidden layer: tanh(x @ cs1.T + b1) in [dh, t] layout, bf16
            hid = [[None] * 2 for _ in range(SC)]
            for sc in range(SC):
                for mo in range(2):
                    pp = psB.tile([128, TC], F32, tag="proj")
                    for dt in range(NDT):
                        nc.tensor.matmul(
                            pp[:],
                            cs1[:, dt * DH + mo * 128: dt * DH + (mo + 1) * 128],
                            xsl(sc, dt),
                            start=(dt == 0), stop=(dt == NDT - 1))
                    hh = hpool.tile([128, TC], BF16, tag=f"h{sc}_{mo}",
                                    name=f"h{sc}_{mo}")
                    nc.scalar.activation(hh[:], pp[:], AF.Tanh,
                                         bias=b1[:, mo:mo + 1])
                    hid[sc][mo] = hh

            # z = hid @ cs2.T + b2  -> [t, h] per (sc, tt); then sigmoid, ln
            zb_all = [[None] * NT for _ in range(SC)]
            for sc in range(SC):
                for tt in range(NT):
                    pp = psT.tile([128, HL], F32, tag="tp")
                    for k2 in range(2):
                        nc.tensor.matmul(
                            pp[:],
                            hid[sc][k2][:, tt * 128:(tt + 1) * 128],
                            cs2[:, k2 * HL:(k2 + 1) * HL],
                            start=(k2 == 0), stop=(k2 == 1))
                    zb = upool.tile([128, HL], F32, tag=f"zb{sc}_{tt}",
                                    name=f"zb{sc}_{tt}")
                    nc.vector.tensor_add(zb[:], pp[:], b2[:])
                    zb_all[sc][tt] = zb
            lc_all = [[None] * NT for _ in range(SC)]
            for sc in range(SC):
                for tt in range(NT):
                    lcv = upool.tile([128, HL], F32, tag=f"lc{sc}_{tt}",
                                     name=f"lc{sc}_{tt}")
                    nc.scalar.activation(lcv[:], zb_all[sc][tt][:], AF.Sigmoid)
                    lc_all[sc][tt] = lcv
            u_all = [[None] * NT for _ in range(SC)]
            for sc in range(SC):
                for tt in range(NT):
                    uu = upool.tile([128, HL], F32, tag=f"u{sc}_{tt}",
                                    name=f"u{sc}_{tt}")
                    nc.scalar.activation(uu[:], lc_all[sc][tt][:], AF.Ln,
                                         bias=1.0, scale=0.2)
                    u_all[sc][tt] = uu

            # =========== per-superchunk compute, software-pipelined =========
            proj_out = {}   # sc -> (q_et, k_et, v_bf)
            rec_mid = {}    # sc -> per-instance intermediates
            ot_tiles = {}   # sc -> OT list

            def emit_proj(sc):
                # Q,K -> per-pair [ (2x64 e), t(TC) ]
                q_et, k_et = [], []
                for name, w, dst in (("q", wq, q_et), ("k", wk, k_et)):
                    for mo in range(NP):
                        pp = psB.tile([128, TC], F32, tag="proj")
                        for dt in range(NDT):
                            nc.tensor.matmul(
                                pp[:],
                                w[:, dt * DL + mo * 128: dt * DL + (mo + 1) * 128],
                                xsl(sc, dt),
                                start=(dt == 0), stop=(dt == NDT - 1))
                        sb = qkv.tile([128, TC], F32R, tag=f"{name}{mo}")
                        nc.vector.tensor_copy(sb[:], pp[:])
                        dst.append(sb)
                # V -> [t(128 x NT), dout(DL)]
                v_te, v_bf = [], []
                for tt in range(NT):
                    pp = psB.tile([128, DL], F32, tag="proj")
                    for dt in range(NDT):
                        nc.tensor.matmul(
                            pp[:, 0:DL],
                            xt[:, dt * T + sc * TC + tt * 128:
                               dt * T + sc * TC + (tt + 1) * 128],
                            wv[:, dt * DL:(dt + 1) * DL],
                            start=(dt == 0), stop=(dt == NDT - 1))
                    sb = qkv.tile([128, DL], F32, tag=f"v{tt}")
                    nc.vector.tensor_copy(sb[:], pp[:, 0:DL])
                    v_te.append(sb)
                    vb = qkv.tile([128, DL], BF16, tag=f"vb{tt}")
                    nc.vector.tensor_copy(vb[:], sb[:])
                    v_bf.append(vb)
                proj_out[sc] = (q_et, k_et, v_te, v_bf)

            def emit_rec_stage1(sc):
                """Decay path + qt/kt/kh for all 8 instances of sc.
                DVE/Act heavy; PE only does the 8 lam transposes."""
                q_et, k_et, v_te, v_bf = proj_out[sc]
                mid = {}
                for tt in range(NT):
                    for mo in range(NP):
                        # log-lambda [t, (2x64 e)] then transpose to pair-et
                        lam = rec.tile([128, 128], F32, tag="lam")
                        for j in range(2):
                            h = 2 * mo + j
                            nc.vector.tensor_scalar(
                                lam[:, j * 64:(j + 1) * 64],
                                lb[:, h * 64:(h + 1) * 64],
                                u_all[sc][tt][:, h:h + 1], LOGCLIP,
                                AL.add, AL.min)
                        lamT = psT.tile([128, 128], F32, tag="tp")
                        nc.tensor.transpose(lamT[:], lam[:], identf[:])
                        L = rec.tile([128, 128], F32, tag="L")
                        nc.vector.tensor_tensor_scan(
                            L[:], lamT[:], z128[:], 0.0, AL.add, AL.add)

                        L127 = L[:, 127:128]
                        ccol = small.tile([128, 1], F32, tag="ccol")
                        nc.vector.tensor_scalar_mul(ccol[:], L127, 0.5)
                        cneg = small.tile([128, 1], F32, tag="cneg")
                        nc.vector.tensor_scalar_mul(cneg[:], L127, -0.5)
                        ec = small.tile([128, 1], F32, tag=f"ec{tt}{mo}")
                        nc.scalar.activation(ec[:], L127, AF.Exp, scale=0.5)
                        aend = small.tile([128, 1], F32, tag=f"ae{tt}{mo}")
                        nc.scalar.activation(aend[:], L127, AF.Exp)

                        eq = rec.tile([128, 128], F32, tag="eq")
                        nc.scalar.activation(eq[:], L[:], AF.Exp, bias=cneg[:])
                        ekc = rec.tile([128, 128], F32, tag="ekc")
                        nc.scalar.activation(ekc[:], L[:], AF.Exp, bias=ccol[:],
                                             scale=-1.0)
                        ek7 = rec.tile([128, 128], F32, tag="ek7")
                        nc.scalar.activation(ek7[:], L[:], AF.Exp, bias=L127,
                                             scale=-1.0)

                        q_p = q_et[mo][:, tt * 128:(tt + 1) * 128]
                        k_p = k_et[mo][:, tt * 128:(tt + 1) * 128]
                        qt = midp.tile([128, 128], BF16, tag=f"qt{tt}{mo}")
                        nc.vector.tensor_mul(qt[:], q_p, eq[:])
                        kt = midp.tile([128, 128], BF16, tag=f"kt{tt}{mo}")
                        nc.vector.tensor_mul(kt[:], k_p, ekc[:])
                        kh = midp.tile([128, 128], F32, tag=f"kh{tt}{mo}")
                        nc.vector.tensor_mul(kh[:], k_p, ek7[:])
                        mid[(tt, mo)] = (qt, kt, kh, ec, aend)
                rec_mid[sc] = mid

            def emit_rec_stage2(sc):
                """PE-heavy: at/khT/sd/op matmuls; DVE masks + state update."""
                q_et, k_et, v_te, v_bf = proj_out[sc]
                mid = rec_mid.pop(sc)
                OT = [otpool.tile([128, TC], BF16, tag=f"ot{mo}",
                                  name=f"OT{mo}_{sc}") for mo in range(NP)]
                ot_tiles[sc] = OT

                # 1) all at matmuls + masks (baseline copy_predicated path)
                atm_all = {}
                for tt in range(NT):
                    for mo in range(NP):
                        qt, kt, kh, ec, aend = mid[(tt, mo)]
                        atm = midp.tile([128, 256], BF16, tag=f"at{tt}{mo}")
                        for j in range(2):
                            sl = slice(j * 64, (j + 1) * 64)
                            at = psAT.tile([128, 128], F32, tag="at4")
                            nc.tensor.matmul(at[:], kt[sl, :], qt[sl, :],
                                             start=True, stop=True)
                            amj = atm[:, j * 128:(j + 1) * 128]
                            nc.gpsimd.tensor_copy(amj, z128b[:])
                            nc.vector.copy_predicated(amj, tri_u8[:], at[:])
                        atm_all[(tt, mo)] = atm

                # 2) khT transposes + sd matmuls
                sd_all = {}
                for tt in range(NT):
                    for mo in range(NP):
                        qt, kt, kh, ec, aend = mid[(tt, mo)]
                        khT = psT.tile([128, 128], F32, tag="tp")
                        nc.tensor.transpose(khT[:], kh[:], identf[:])
                        khTs = midp.tile([128, 128], F32, tag=f"kT{tt}{mo}")
                        nc.vector.tensor_copy(khTs[:], khT[:])
                        sd_all[(tt, mo)] = khTs

                # 3) per chunk: ssc, sd matmul, op matmuls, state update
                for tt in range(NT):
                    for mo in range(NP):
                        qt, kt, kh, ec, aend = mid[(tt, mo)]
                        v_p = v_te[tt][:, mo * 128:(mo + 1) * 128]
                        khTs = sd_all[(tt, mo)]
                        atm = atm_all[(tt, mo)]

                        ssc = midp.tile([128, 64], BF16, tag=f"sc{tt}{mo}")
                        nc.vector.tensor_scalar_mul(ssc[:], S[mo][:], ec[:])

                        sd = psSD.tile([128, 128], F32, tag="sd")
                        nc.tensor.matmul(sd[:], khTs[:], v_p, start=True,
                                         stop=True)

                        op = psOP.tile([128, 128], F32, tag="outT")
                        for j in range(2):
                            sl = slice(j * 64, (j + 1) * 64)
                            nc.tensor.matmul(op[sl, :],
                                             v_bf[tt][:, mo * 128 + j * 64:
                                                      mo * 128 + (j + 1) * 64],
                                             atm[:, j * 128:(j + 1) * 128],
                                             start=True, stop=False)
                            nc.tensor.matmul(op[sl, :], ssc[sl, :], qt[sl, :],
                                             start=False, stop=True)
                            nc.vector.scalar_tensor_tensor(
                                S[mo][sl, :], S[mo][sl, :], aend[sl, :],
                                sd[sl, j * 64:(j + 1) * 64], AL.mult, AL.add)
                        nc.vector.tensor_copy(
                            OT[mo][:, tt * 128:(tt + 1) * 128], op[:])

            def emit_oproj(sc):
                OT = ot_tiles.pop(sc)
                for tt in range(NT):
                    for no in range(2):
                        pp = psB.tile([128, 512], F32, tag="proj")
                        for mo in range(NP):
                            nc.tensor.matmul(
                                pp[:],
                                OT[mo][:, tt * 128:(tt + 1) * 128],
                                wo[:, mo * D + no * 512: mo * D + no * 512 + 512],
                                start=(mo == 0), stop=(mo == NP - 1))
                        ysb = ypool.tile([128, 512], F32, tag="y")
                        nc.scalar.copy(ysb[:], pp[:])
                        nc.sync.dma_start(
                            y_d[sc * TC + tt * 128: sc * TC + (tt + 1) * 128,
                                no * 512:(no + 1) * 512],
                            ysb[:])

            # pipeline: proj(sc) overlaps recurrence elementwise of sc-1
            emit_proj(0)
            for sc in range(1, SC):
                emit_rec_stage1(sc - 1)
                emit_proj(sc)
                emit_rec_stage2(sc - 1)
                emit_oproj(sc - 1)
            emit_rec_stage1(SC - 1)
            emit_rec_stage2(SC - 1)
            emit_oproj(SC - 1)

    _split_multiwait(nc)
    return nc


_NC = None
LAST = None  # last BassKernelResults (exec_time_ns, trace path) for test harness

def _get_nc():
    global _NC
    if _NC is None:
        _NC = _build()
    return _NC


def _sigmoid(x):
    return 1.0 / (1.0 + np.exp(-x))


def kernel(x, q_w, k_w, v_w, o_w, cs_w1, cs_b1, cs_w2, cs_b2, decay_params):
    x = np.asarray(x, np.float32)
    nc = _get_nc()
    bf16 = mybir.dt.np(BF16)

    def wlay(wT_cols):  # [1024, M] -> [128, 8*M] (dt-major along free)
        return np.ascontiguousarray(
            wT_cols.reshape(NDT, 128, wT_cols.shape[1]).transpose(1, 0, 2)
            .reshape(128, -1))

    qwT = np.asarray(q_w, np.float32).T
    kwT = np.asarray(k_w, np.float32).T
    vwT = np.asarray(v_w, np.float32).T
    owT = np.asarray(o_w, np.float32).T
    cs1T = np.asarray(cs_w1, np.float32).T      # [1024, 256]
    cs2T = np.asarray(cs_w2, np.float32).T      # [256, 16]
    lbase = np.log(_sigmoid(np.asarray(decay_params, np.float32)))  # [H, E]
    b1c = np.ascontiguousarray(np.asarray(cs_b1, np.float32).reshape(2, 128).T)

    in_maps = []
    for i in range(8):
        b, g = i // 2, i % 2
        hs = g * HL
        xT = x[b].T                                            # [1024, 2048]
        xTl = np.ascontiguousarray(
            xT.reshape(NDT, 128, T).transpose(1, 0, 2).reshape(128, NDT * T))
        wo_loc = owT[hs * E:(hs + HL) * E, :]                  # [512, 1024]
        wol = np.ascontiguousarray(                            # [128, 4*1024]
            wo_loc.reshape(4, 128, D).transpose(1, 0, 2).reshape(128, 4 * D))
        cs2l = np.ascontiguousarray(
            cs2T[:, hs:hs + HL].reshape(2, 128, HL).transpose(1, 0, 2)
            .reshape(128, 2 * HL))
        in_maps.append({
            "xT": xTl.astype(bf16),
            "wq": wlay(qwT[:, hs * E:(hs + HL) * E]).astype(bf16),
            "wk": wlay(kwT[:, hs * E:(hs + HL) * E]).astype(bf16),
            "wv": wlay(vwT[:, hs * E:(hs + HL) * E]).astype(bf16),
            "wo": wol.astype(bf16),
            "cs1": wlay(cs1T).astype(bf16),
            "cs2": cs2l.astype(bf16),
            "b1": b1c,
            "b2": np.ascontiguousarray(
                np.broadcast_to(np.asarray(cs_b2, np.float32)[hs:hs + HL],
                                (128, HL))),
            "lb": np.ascontiguousarray(
                np.broadcast_to(lbase[hs:hs + HL].reshape(1, DL), (128, DL))),
        })

    res = run_bass_kernel_spmd(nc, in_maps, core_ids=list(range(8)))
    global LAST
    LAST = res
    y = np.empty((B, T, D), np.float32)
    for b in range(B):
        y[b] = res.results[2 * b]["y"] + res.results[2 * b + 1]["y"]
    return y
